# revision 8
# baseline (speedup 1.0000x reference)
"""NT-Xent / SimCLR contrastive loss on 8 Trainium2 NeuronCores.

Math (matches the jax reference):
    z = l2_normalize(concat([emb_i, emb_j]))          # [2B, D] unit rows
    sim = z @ z.T                                     # cosine similarities
    denom_r = sum_{j != r} exp(sim_rj / T)
    pos_r   = z_r . z_{(r+B) mod 2B}                  # the positive pair
    loss = mean_r( log(denom_r) - pos_r / T )

Sharding: the 2B=8192 rows are data-parallel over 8 cores. Each core
receives the full representation matrix ROTATED by -c*1024 rows, so its
slab is always local rows 0..1023 (one SPMD program for all cores).
Row sums are invariant to column order, and the positive partner of
local row r is always local row r+4096 (the roll is half the cycle).

Per-core pipeline (v2 — engine-balanced):
  - 1MiB strided loads on the gpsimd SWDGE queues
  - sum-of-squares via one bn_stats pass per 8-tile group (DVE),
    rsqrt as exp(-0.5*ln(x)) on ACT (Ln/Exp share one table set)
  - normalize+cast bf16 on gpsimd (tensor_scalar, idle engine)
  - transpose to [D, 8192]: tiles 0..31 on the PE (is_transpose via an
    inline identity, scoped PSUM pool closed before the main loop),
    tiles 32..63 on the two HWDGE xbar rings (sync + scalar engines)
  - 256 bf16 matmuls [128x512] accumulate [128, 2048] PSUM blocks;
    ACT computes exp(2*sim) straight out of PSUM with a fused row
    accumulator (the similarity matrix never reaches HBM)
  - exact diagonal e^2 subtracted; positives computed in fp32;
    each core returns [128, 8] per-row partials; host sums / 2B.
"""

import numpy as np
from contextlib import ExitStack

import ml_dtypes
import concourse.bass as bass
import concourse.tile as tile
from concourse import bacc, mybir
from concourse._compat import with_exitstack
from concourse.bass_utils import run_bass_kernel_spmd

B = 4096
D = 256
R = 2 * B
N_CORES = 8
SLAB = R // N_CORES
INV_T = 2.0
E2 = float(np.exp(2.0))

F32 = mybir.dt.float32
BF16 = mybir.dt.bfloat16

NT = R // 128          # 64 row tiles
NG = 8                 # load groups (8 tiles each)
M_TILES = SLAB // 128  # 8
NB = 4                 # psum blocks of 2048 cols
N_PE_T = 32            # tiles transposed on the PE (rest on xbar rings)


@with_exitstack
def _loss_kernel(ctx: ExitStack, tc: "tile.TileContext", out_ap: bass.AP,
                 reps_ap: bass.AP, ident_ap: bass.AP):
    nc = tc.nc
    mult = mybir.AluOpType.mult
    add = mybir.AluOpType.add
    Exp = mybir.ActivationFunctionType.Exp
    Ln = mybir.ActivationFunctionType.Ln

    xpool = ctx.enter_context(tc.tile_pool(name="x", bufs=NG))
    stats = ctx.enter_context(tc.tile_pool(name="stats", bufs=2))
    scales = ctx.enter_context(tc.tile_pool(name="scales", bufs=2))
    zpool = ctx.enter_context(tc.tile_pool(name="z16", bufs=6))
    z32pool = ctx.enter_context(tc.tile_pool(name="z32", bufs=4))
    prodpool = ctx.enter_context(tc.tile_pool(name="prod", bufs=2))
    rtpool = ctx.enter_context(tc.tile_pool(name="repsT", bufs=16))
    cpool = ctx.enter_context(tc.tile_pool(name="const", bufs=1))
    epool = ctx.enter_context(tc.tile_pool(name="escratch", bufs=2))
    accpool = ctx.enter_context(tc.tile_pool(name="acc", bufs=1))
    fpool = ctx.enter_context(tc.tile_pool(name="final", bufs=1))

    ident = cpool.tile([128, 128], BF16, tag="ident")
    nc.sync.dma_start(ident[:], ident_ap[:])

    # ---- loads (gpsimd SWDGE) -------------------------------------------
    xg = []
    for g in range(NG):
        xt = xpool.tile([128, NG, D], F32, tag="x", name=f"x{g}")
        src = reps_ap[g * 1024:(g + 1) * 1024, :].rearrange("(t p) d -> p t d", p=128)
        nc.gpsimd.dma_start(xt[:], src)
        xg.append(xt)

    rts = [
        rtpool.tile([128, 4, 2, 128], BF16, tag="repsT", name=f"repsT{j}")
        for j in range(16)
    ]

    # ---- sum-of-squares via bn_stats, scales via Ln/Exp (2 halves) ------
    # bn_stats: [c_e, mean_e, c_e*var_e, c_o, mean_o, c_o*var_o] per tile;
    # sumsq = cv_e + cv_o + 128*(mean_e^2 + mean_o^2)
    scale_h = []
    for h in range(2):
        st = stats.tile([128, 32, 6], F32, tag="stats", name=f"stats{h}")
        for g4 in range(4):
            g = h * 4 + g4
            for tl in range(8):  # one stats-tuple per instruction
                nc.vector.bn_stats(
                    st[:, g4 * 8 + tl, :], xg[g][:, tl, :],
                )
        means = st[:, :, 1::3]
        cvs = st[:, :, 2::3]
        msq = stats.tile([128, 32, 2], F32, tag="msq", name=f"msq{h}")
        nc.vector.tensor_mul(msq[:], means, means)
        s2 = stats.tile([128, 32, 2], F32, tag="s2", name=f"s2{h}")
        nc.vector.tensor_scalar(
            out=s2[:], in0=msq[:], scalar1=128.0, scalar2=None, op0=mult,
        )
        nc.vector.tensor_add(s2[:], s2[:], cvs)
        ssq = stats.tile([128, 32], F32, tag="ssq", name=f"ssq{h}")
        nc.vector.tensor_reduce(
            out=ssq[:], in_=s2[:], axis=mybir.AxisListType.X, op=add,
        )
        lnv = stats.tile([128, 32], F32, tag="lnv", name=f"lnv{h}")
        sc = scales.tile([128, 32], F32, tag="scale", name=f"sc{h}")
        nc.scalar.activation(lnv[:], ssq[:], Ln)
        nc.scalar.activation(sc[:], lnv[:], Exp, scale=-0.5)
        scale_h.append(sc)

    # ---- normalize (gpsimd) + transpose (PE early / xbar rings late) ----
    with tc.tile_pool(name="tpsum", bufs=4, space="PSUM") as tpsum:
        ring = [nc.sync, nc.scalar]
        for t in range(NT):
            g, tl = t // NG, t % NG
            h, hc = t // 32, t % 32
            z16 = zpool.tile([128, D], BF16, tag="z16", name=f"z{t}")
            nc.gpsimd.tensor_scalar(
                out=z16[:], in0=xg[g][:, tl, :],
                scalar1=scale_h[h][:, hc:hc + 1], scalar2=None, op0=mult,
            )
            j, tj = t // 4, t % 4
            for k in range(2):
                dst = rts[j][:, tj, k, :]
                src = z16[:, k * 128:(k + 1) * 128]
                if t < N_PE_T:
                    tp = tpsum.tile([128, 128], BF16, tag="tp", name=f"tp{t}_{k}")
                    nc.tensor.transpose(tp[:], src, ident[:])
                    nc.vector.tensor_copy(dst, tp[:])
                else:
                    ring[t % 2].dma_start_transpose(dst, src)

        # ---- positives (fp32; za carries the -1/T factor) ---------------
        posneg = accpool.tile([128, M_TILES], F32, tag="posneg")
        for i in range(M_TILES):
            za = z32pool.tile([128, D], F32, tag="z32", name=f"za{i}")
            nc.gpsimd.tensor_scalar(
                out=za[:], in0=xg[0][:, i, :],
                scalar1=scale_h[0][:, i:i + 1], scalar2=-INV_T, op0=mult, op1=mult,
            )
            zb = z32pool.tile([128, D], F32, tag="z32", name=f"zb{i}")
            nc.gpsimd.tensor_scalar(
                out=zb[:], in0=xg[4][:, i, :],
                scalar1=scale_h[1][:, i:i + 1], scalar2=None, op0=mult,
            )
            prod = prodpool.tile([128, D], F32, tag="prod", name=f"pp{i}")
            nc.vector.tensor_mul(prod[:], za[:], zb[:])
            nc.vector.tensor_reduce(
                out=posneg[:, i:i + 1], in_=prod[:],
                axis=mybir.AxisListType.X, op=add,
            )

    # ---- similarity slab + fused exp/rowsum -----------------------------
    psum = ctx.enter_context(tc.tile_pool(name="mm", bufs=2, space="PSUM"))
    denacc = accpool.tile([128, M_TILES * NB], F32, tag="denacc")
    for nb in range(NB):
        for m in range(M_TILES):
            jm, tm = m // 4, m % 4
            pt = psum.tile([128, 2048], F32, tag="mm", name=f"pt{nb}_{m}")
            for ns in range(4):
                j = nb * 4 + ns
                for k in range(2):
                    nc.tensor.matmul(
                        pt[:, ns * 512:(ns + 1) * 512],
                        lhsT=rts[jm][:, tm, k, :],
                        rhs=rts[j][:, :, k, :],
                        start=(k == 0),
                        stop=(k == 1),
                    )
            esc = epool.tile([128, 2048], BF16, tag="esc", name=f"esc{nb}_{m}")
            nc.scalar.activation(
                esc[:], pt[:], Exp, scale=INV_T,
                accum_out=denacc[:, m * NB + nb:m * NB + nb + 1],
            )

    # ---- finalize -------------------------------------------------------
    drow = fpool.tile([128, M_TILES], F32, tag="drow")
    nc.vector.tensor_reduce(
        out=drow[:],
        in_=denacc[:].rearrange("p (m n) -> p m n", n=NB),
        axis=mybir.AxisListType.X,
        op=add,
    )
    dcorr = fpool.tile([128, M_TILES], F32, tag="dcorr")
    nc.vector.tensor_scalar(
        out=dcorr[:], in0=drow[:], scalar1=-E2, scalar2=None, op0=add,
    )
    ld = fpool.tile([128, M_TILES], F32, tag="ld")
    nc.scalar.activation(ld[:], dcorr[:], Ln)
    loss = fpool.tile([128, M_TILES], F32, tag="loss")
    nc.vector.tensor_add(loss[:], ld[:], posneg[:])
    nc.sync.dma_start(out_ap[:], loss[:])


_CACHE = {}


def _get_compiled():
    if "nc" not in _CACHE:
        nc = bacc.Bacc("TRN2", target_bir_lowering=False, debug=False)
        reps_in = nc.dram_tensor("reps", [R, D], F32, kind="ExternalInput")
        ident_t = nc.inline_tensor(
            np.eye(128).astype(ml_dtypes.bfloat16), name="ident"
        )
        part_out = nc.dram_tensor("partial", [128, M_TILES], F32, kind="ExternalOutput")
        with tile.TileContext(nc) as tc:
            _loss_kernel(tc, part_out.ap(), reps_in.ap(), ident_t.ap())
        nc.compile()
        _CACHE["nc"] = nc
    return _CACHE["nc"]


def make_in_maps(emb_i: np.ndarray, emb_j: np.ndarray):
    reps = np.concatenate(
        [np.asarray(emb_i, dtype=np.float32), np.asarray(emb_j, dtype=np.float32)],
        axis=0,
    )
    return [
        {"reps": np.ascontiguousarray(np.roll(reps, -c * SLAB, axis=0))}
        for c in range(N_CORES)
    ]


def run_spmd(emb_i, emb_j, **kwargs):
    nc = _get_compiled()
    in_maps = make_in_maps(emb_i, emb_j)
    return run_bass_kernel_spmd(nc, in_maps, core_ids=list(range(N_CORES)), **kwargs)


def kernel(emb_i: np.ndarray, emb_j: np.ndarray) -> np.ndarray:
    res = run_spmd(emb_i, emb_j)
    total = 0.0
    for c in range(N_CORES):
        total += float(np.sum(res.results[c]["partial"].astype(np.float64)))
    return np.array(total / R, dtype=np.float32)


# revision 12
# speedup vs baseline: 2.1637x; 2.1637x over previous
"""NT-Xent / SimCLR contrastive loss on 8 Trainium2 NeuronCores.

Math (matches the jax reference):
    z = l2_normalize(concat([emb_i, emb_j]))          # [2B, D] unit rows
    sim = z @ z.T                                     # cosine similarities
    denom_r = sum_{j != r} exp(sim_rj / T)
    pos_r   = z_r . z_{(r+B) mod 2B}                  # the positive pair
    loss = mean_r( log(denom_r) - pos_r / T )

Sharding: the 2B=8192 rows are data-parallel over 8 cores. Each core
receives the full representation matrix ROTATED by -c*1024 rows, so its
slab is always local rows 0..1023 (one SPMD program for all cores).
Row sums are invariant to column order, and the positive partner of
local row r is always local row r+4096 (the roll is half the cycle).

Per-core pipeline (v2 — engine-balanced):
  - 1MiB strided loads on the gpsimd SWDGE queues
  - sum-of-squares via one bn_stats pass per 8-tile group (DVE),
    rsqrt as exp(-0.5*ln(x)) on ACT (Ln/Exp share one table set)
  - normalize+cast bf16 on gpsimd (tensor_scalar, idle engine)
  - transpose to [D, 8192]: tiles 0..31 on the PE (is_transpose via an
    inline identity, scoped PSUM pool closed before the main loop),
    tiles 32..63 on the two HWDGE xbar rings (sync + scalar engines)
  - 256 bf16 matmuls [128x512] accumulate [128, 2048] PSUM blocks;
    ACT computes exp(2*sim) straight out of PSUM with a fused row
    accumulator (the similarity matrix never reaches HBM)
  - exact diagonal e^2 subtracted; positives computed in fp32;
    each core returns [128, 8] per-row partials; host sums / 2B.
"""

import numpy as np
from contextlib import ExitStack

import ml_dtypes
import concourse.bass as bass
import concourse.tile as tile
from concourse import bacc, mybir
from concourse._compat import with_exitstack
from concourse.bass_utils import run_bass_kernel_spmd

B = 4096
D = 256
R = 2 * B
N_CORES = 8
SLAB = R // N_CORES
INV_T = 2.0
E2 = float(np.exp(2.0))

F32 = mybir.dt.float32
BF16 = mybir.dt.bfloat16

NT = R // 128          # 64 row tiles
NG = 8                 # load groups (8 tiles each)
M_TILES = SLAB // 128  # 8
NB = 4                 # psum blocks of 2048 cols
N_PE_T = 20            # tiles transposed on the PE (rest on xbar rings)


@with_exitstack
def _loss_kernel(ctx: ExitStack, tc: "tile.TileContext", out_ap: bass.AP,
                 reps_ap: bass.AP, ident_ap: bass.AP):
    nc = tc.nc
    mult = mybir.AluOpType.mult
    add = mybir.AluOpType.add
    Exp = mybir.ActivationFunctionType.Exp
    Ln = mybir.ActivationFunctionType.Ln

    xpool = ctx.enter_context(tc.tile_pool(name="x", bufs=NG))
    stats = ctx.enter_context(tc.tile_pool(name="stats", bufs=2))
    scales = ctx.enter_context(tc.tile_pool(name="scales", bufs=2))
    zpool = ctx.enter_context(tc.tile_pool(name="z16", bufs=6))
    z32pool = ctx.enter_context(tc.tile_pool(name="z32", bufs=4))
    prodpool = ctx.enter_context(tc.tile_pool(name="prod", bufs=2))
    rtpool = ctx.enter_context(tc.tile_pool(name="repsT", bufs=16))
    cpool = ctx.enter_context(tc.tile_pool(name="const", bufs=1))
    epool = ctx.enter_context(tc.tile_pool(name="escratch", bufs=2))
    accpool = ctx.enter_context(tc.tile_pool(name="acc", bufs=1))
    fpool = ctx.enter_context(tc.tile_pool(name="final", bufs=1))

    ident = cpool.tile([128, 128], F32, tag="ident")
    nc.sync.dma_start(ident[:], ident_ap[:])

    # ---- loads (gpsimd SWDGE) -------------------------------------------
    xg = []
    for g in range(NG):
        xt = xpool.tile([128, NG, D], F32, tag="x", name=f"x{g}")
        src = reps_ap[g * 1024:(g + 1) * 1024, :].rearrange("(t p) d -> p t d", p=128)
        nc.gpsimd.dma_start(xt[:], src)
        xg.append(xt)

    rts = [
        rtpool.tile([128, 4, 2, 128], BF16, tag="repsT", name=f"repsT{j}")
        for j in range(16)
    ]

    # ---- sum-of-squares via fused square+row-accumulate (2 halves) ------
    scale_h = []
    for h in range(2):
        ssq = stats.tile([128, 32], F32, tag="ssq", name=f"ssq{h}")
        for g4 in range(4):
            g = h * 4 + g4
            for tl in range(8):
                junk = prodpool.tile([128, D], F32, tag="prod", name=f"sq{g}_{tl}")
                nc.vector.scalar_tensor_tensor(
                    out=junk[:], in0=xg[g][:, tl, :], scalar=1.0,
                    in1=xg[g][:, tl, :], op0=mult, op1=mult,
                    accum_out=ssq[:, g4 * 8 + tl:g4 * 8 + tl + 1],
                )
        lnv = stats.tile([128, 32], F32, tag="lnv", name=f"lnv{h}")
        sc = scales.tile([128, 32], F32, tag="scale", name=f"sc{h}")
        nc.scalar.activation(lnv[:], ssq[:], Ln)
        nc.scalar.activation(sc[:], lnv[:], Exp, scale=-0.5)
        scale_h.append(sc)

    # ---- normalize (DVE) + transpose (PE early / xbar rings late) -------
    with tc.tile_pool(name="tpsum", bufs=4, space="PSUM") as tpsum:
        ring = [nc.sync, nc.scalar]
        for t in range(NT):
            g, tl = t // NG, t % NG
            h, hc = t // 32, t % 32
            on_pe = t < N_PE_T
            # PE-path tiles stay fp32 (2x DVE mode; the PSUM copy casts);
            # xbar-path tiles must be bf16 (2-byte xbar restriction).
            z16 = zpool.tile([128, D], F32 if on_pe else BF16,
                             tag="zpe" if on_pe else "z16", name=f"z{t}")
            nc.vector.tensor_scalar(
                out=z16[:], in0=xg[g][:, tl, :],
                scalar1=scale_h[h][:, hc:hc + 1], scalar2=None, op0=mult,
            )
            j, tj = t // 4, t % 4
            for k in range(2):
                dst = rts[j][:, tj, k, :]
                src = z16[:, k * 128:(k + 1) * 128]
                if on_pe:
                    tp = tpsum.tile([128, 128], F32, tag="tp", name=f"tp{t}_{k}")
                    nc.tensor.transpose(tp[:], src, ident[:])
                    nc.vector.tensor_copy(dst, tp[:])
                else:
                    ring[t % 2].dma_start_transpose(dst, src)

        # ---- positives (fp32; the -1/T factor rides on the scale) -------
        posneg = accpool.tile([128, M_TILES], F32, tag="posneg")
        scm2 = scales.tile([128, M_TILES], F32, tag="scm2")
        nc.vector.tensor_scalar(
            out=scm2[:], in0=scale_h[0][:, 0:M_TILES],
            scalar1=-INV_T, scalar2=None, op0=mult,
        )
        for i in range(M_TILES):
            zb = z32pool.tile([128, D], F32, tag="z32", name=f"zb{i}")
            nc.vector.tensor_scalar(
                out=zb[:], in0=xg[4][:, i, :],
                scalar1=scale_h[1][:, i:i + 1], scalar2=None, op0=mult,
            )
            prod = prodpool.tile([128, D], F32, tag="prod", name=f"pp{i}")
            nc.vector.scalar_tensor_tensor(
                out=prod[:], in0=xg[0][:, i, :], scalar=scm2[:, i:i + 1],
                in1=zb[:], op0=mult, op1=mult,
                accum_out=posneg[:, i:i + 1],
            )

    # ---- similarity slab + fused exp/rowsum -----------------------------
    psum = ctx.enter_context(tc.tile_pool(name="mm", bufs=2, space="PSUM"))
    denacc = accpool.tile([128, M_TILES * NB], F32, tag="denacc")
    for nb in range(NB):
        for m in range(M_TILES):
            jm, tm = m // 4, m % 4
            pt = psum.tile([128, 2048], F32, tag="mm", name=f"pt{nb}_{m}")
            for ns in range(4):
                j = nb * 4 + ns
                for k in range(2):
                    nc.tensor.matmul(
                        pt[:, ns * 512:(ns + 1) * 512],
                        lhsT=rts[jm][:, tm, k, :],
                        rhs=rts[j][:, :, k, :],
                        start=(k == 0),
                        stop=(k == 1),
                    )
            esc = epool.tile([128, 2048], BF16, tag="esc", name=f"esc{nb}_{m}")
            nc.scalar.activation(
                esc[:], pt[:], Exp, scale=INV_T,
                accum_out=denacc[:, m * NB + nb:m * NB + nb + 1],
            )

    # ---- finalize -------------------------------------------------------
    drow = fpool.tile([128, M_TILES], F32, tag="drow")
    nc.vector.tensor_reduce(
        out=drow[:],
        in_=denacc[:].rearrange("p (m n) -> p m n", n=NB),
        axis=mybir.AxisListType.X,
        op=add,
    )
    dcorr = fpool.tile([128, M_TILES], F32, tag="dcorr")
    nc.vector.tensor_scalar(
        out=dcorr[:], in0=drow[:], scalar1=-E2, scalar2=None, op0=add,
    )
    ld = fpool.tile([128, M_TILES], F32, tag="ld")
    nc.scalar.activation(ld[:], dcorr[:], Ln)
    loss = fpool.tile([128, M_TILES], F32, tag="loss")
    nc.vector.tensor_add(loss[:], ld[:], posneg[:])
    nc.sync.dma_start(out_ap[:], loss[:])


_CACHE = {}


def _get_compiled():
    if "nc" not in _CACHE:
        nc = bacc.Bacc("TRN2", target_bir_lowering=False, debug=False)
        reps_in = nc.dram_tensor("reps", [R, D], F32, kind="ExternalInput")
        ident_t = nc.inline_tensor(np.eye(128, dtype=np.float32), name="ident")
        part_out = nc.dram_tensor("partial", [128, M_TILES], F32, kind="ExternalOutput")
        with tile.TileContext(nc) as tc:
            _loss_kernel(tc, part_out.ap(), reps_in.ap(), ident_t.ap())
        nc.compile()
        _CACHE["nc"] = nc
    return _CACHE["nc"]


def make_in_maps(emb_i: np.ndarray, emb_j: np.ndarray):
    reps = np.concatenate(
        [np.asarray(emb_i, dtype=np.float32), np.asarray(emb_j, dtype=np.float32)],
        axis=0,
    )
    return [
        {"reps": np.ascontiguousarray(np.roll(reps, -c * SLAB, axis=0))}
        for c in range(N_CORES)
    ]


def run_spmd(emb_i, emb_j, **kwargs):
    nc = _get_compiled()
    in_maps = make_in_maps(emb_i, emb_j)
    return run_bass_kernel_spmd(nc, in_maps, core_ids=list(range(N_CORES)), **kwargs)


def kernel(emb_i: np.ndarray, emb_j: np.ndarray) -> np.ndarray:
    res = run_spmd(emb_i, emb_j)
    total = 0.0
    for c in range(N_CORES):
        total += float(np.sum(res.results[c]["partial"].astype(np.float64)))
    return np.array(total / R, dtype=np.float32)
